# revision 1
# baseline (speedup 1.0000x reference)
"""Multigrid F-cycle advection smoother on 8 TRN2 NeuronCores.

Strategy (zero-communication, redundant compute):
  - Column-shard: core k computes a [4096, 640] window of u:
    core 0 -> cols [0, 640), core k>0 -> cols [512k-128, 512k+512).
    Host keeps cols [0:512) from core 0, [128:640) from others.
    Window-left boundary garbage contaminates <= 31 cols/step (124 < 128 halo).
  - On-chip layout: partition p holds rows [32p, 32p+32); free = [row_lo, col].
    Up-shift = free offset, left-shift = free offset except col 0.
  - All intermediates f16 (validated: max rel err ~2.2e-3 vs f32 reference).
  - Per step: T = u_l + u_u (clamped); pure pair-sum restriction chain S1..S5
    (S1 rescaled by 1/256 to keep f16 range); error chain
    E_j = P + lam*(Pl0+Pu0) + alpha_j*S_j + gamma_j with zero boundaries
    (gamma injected via the (Pl0 + g) term); u += 0.1/DIAG*T; u -= prol(E1~).
"""
import numpy as np

import concourse.bass as bass
import concourse.bacc as bacc
import concourse.mybir as mybir
from concourse import tile
from concourse.bass_utils import run_bass_kernel_spmd

F32 = mybir.dt.float32
F16 = mybir.dt.float16
ADD = mybir.AluOpType.add
MULT = mybir.AluOpType.mult

NROW, WCOL, OWN, HALO = 4096, 640, 512, 128
DIAG = 1.2
LAM = float(np.float32(0.1 / DIAG))
SQ1 = float(np.float32(0.1 / DIAG))
RS = 1.0 / 256.0
ALPHA = {j: float(np.float32(-0.1 * 4.0 ** (-j) / DIAG)) for j in range(1, 6)}
ALPHA_S = {j: float(np.float32(ALPHA[j] / RS)) for j in range(1, 6)}

_NC_CACHE = {}


def _v3(ap, c):
    return ap.rearrange("p (r c) -> p r c", c=c)


def _build(t_steps: int):
    nc = bacc.Bacc(None, target_bir_lowering=False)
    u_ext = nc.declare_dram_parameter("u", [NROW, WCOL], F32, isOutput=False)
    c_ext = nc.declare_dram_parameter("consts", [128, 8], F32, isOutput=False)
    out_ext = nc.declare_dram_parameter("out", [NROW, WCOL], F32, isOutput=True)

    u_dram = u_ext[:, :].rearrange("(p r) c -> p r c", p=128)
    out_dram = out_ext[:, :].rearrange("(p r) c -> p r c", p=128)

    DVE, GPS, ACT = nc.vector, nc.gpsimd, nc.scalar

    with tile.TileContext(nc) as tc:
        with tc.tile_pool(name="sb", bufs=1) as pool:
            # u: up-halo row 0 (even stride 640 keeps 2x alignment); data rows 1..32
            ua = pool.tile([128, 33 * 640], F16)
            T = pool.tile([128, 32 * 640], F16)
            EC = pool.tile([128, 16 * 640], F16)
            S = {1: pool.tile([128, 16 * 320], F16, name="s1")}
            rp = {}
            for j in range(2, 6):
                rl, cl = 32 >> j, 640 >> j
                rp[j] = pool.tile([128, max(2 * rl * cl, 8)], F16, name=f"rp{j}")
                S[j] = pool.tile([128, max(rl * cl, 8)], F16, name=f"s{j}")
            E = {j: pool.tile([128, max((32 >> j) * (640 >> j), 8)], F16, name=f"e{j}")
                 for j in range(1, 6)}
            # P: up-halo row 0 only; data rows 1..rl (even col stride)
            P = {j: pool.tile([128, (1 + (32 >> j)) * (640 >> j)], F16, name=f"pp{j}")
                 for j in range(1, 5)}
            consts = pool.tile([128, 8], F32)

            nc.sync.dma_start(consts[:, :], c_ext[:, :])
            uav = _v3(ua[:, :], 640)
            GPS.dma_start(uav[:, 1:33, :], u_dram)
            # P partition-0 halo row = zero forever (global top boundary)
            for j in range(1, 5):
                DVE.memset(_v3(P[j][:, :], 640 >> j)[0:1, 0:1, :], 0.0)

            g_ap = {j: consts[:, j - 1:j] for j in range(1, 5)}
            gam5_ap = consts[:, 4:5]

            for _step in range(t_steps):
                # ---- u up-halo row (prev partition last data row; part0 clamps row0)
                nc.sync.dma_start(uav[1:128, 0, :], uav[0:127, 32, :])
                ACT.copy(uav[0:1, 0, :], uav[0:1, 1, :])

                # ---- T = u_left + u_up ----
                T3 = _v3(T[:, :], 640)
                DVE.tensor_tensor(T3[:, :, 1:640], uav[:, 1:33, 0:639], uav[:, 0:32, 1:640], ADD)
                DVE.tensor_tensor(T3[:, :, 0:1], uav[:, 1:33, 0:1], uav[:, 0:32, 0:1], ADD)

                # ---- restriction: RP -> rescale -> S1 (GPS colpair) ----
                RP = pool.tile([128, 16 * 640], F16, name=f"rps{_step}", tag="scr")
                Tp = T[:, :].rearrange("p (rp two c) -> p rp two c", rp=16, two=2, c=640)
                RP3 = _v3(RP, 640)
                DVE.tensor_tensor(RP3[:, :, :], Tp[:, :, 0, :], Tp[:, :, 1, :], ADD)
                DVE.tensor_scalar(RP, RP, RS, None, MULT)
                RPc = RP.rearrange("p (r cp two) -> p r cp two", r=16, two=2)
                S13 = _v3(S[1][:, :], 320)
                GPS.tensor_tensor(S13[:, :, :], RPc[:, :, :, 0], RPc[:, :, :, 1], ADD)

                prev = S[1]
                for j in range(2, 6):
                    rl, cl = 32 >> j, 640 >> j
                    pv = prev[:, :].rearrange("p (rp two c) -> p rp two c", rp=rl, two=2, c=2 * cl)
                    rp3 = _v3(rp[j][:, 0:2 * rl * cl], 2 * cl)
                    GPS.tensor_tensor(rp3[:, :, :], pv[:, :, 0, :], pv[:, :, 1, :], ADD)
                    rpc = rp[j][:, 0:2 * rl * cl].rearrange("p (r cp two) -> p r cp two", r=rl, two=2)
                    s3 = _v3(S[j][:, 0:rl * cl], cl)
                    GPS.tensor_tensor(s3[:, :, :], rpc[:, :, :, 0], rpc[:, :, :, 1], ADD)
                    prev = S[j]

                # ---- scale S_j in place ----
                DVE.tensor_scalar(S[1][:, :], S[1][:, :], ALPHA_S[1], None, MULT)
                for j in range(2, 5):
                    rl, cl = 32 >> j, 640 >> j
                    DVE.tensor_scalar(S[j][:, 0:rl * cl], S[j][:, 0:rl * cl], ALPHA_S[j], None, MULT)

                # ---- E chain ----
                DVE.tensor_scalar(E[5][:, 0:20], S[5][:, 0:20], ALPHA_S[5], gam5_ap, MULT, ADD)

                for j in range(4, 0, -1):
                    rl, cl = 32 >> j, 640 >> j
                    Ein3 = _v3(E[j + 1][:, 0:(rl // 2) * (cl // 2)], cl // 2)
                    Edup = Ein3.unsqueeze(3).broadcast_to([128, rl // 2, cl // 2, 2])
                    Pv = _v3(P[j][:, :], cl)
                    Pd = Pv[:, 1:rl + 1, :]
                    Pr = Pd.rearrange("p (rp two) c -> p rp two c", rp=rl // 2, two=2) \
                           .rearrange("p rp two (cp ctwo) -> p rp two cp ctwo", cp=cl // 2, ctwo=2)
                    ACT.copy(Pr[:, :, 0, :, :], Edup)
                    ACT.copy(Pr[:, :, 1, :, :], Edup)
                    nc.sync.dma_start(Pv[1:128, 0, :], Pv[0:127, rl, :])

                    Ej = E[j][:, 0:rl * cl]
                    Ej3 = _v3(Ej, cl)
                    # E = Pl0 + Pu0 (col 0: Pl0 = 0 -> copy strip)
                    DVE.tensor_tensor(Ej3[:, :, 1:cl], Pv[:, 1:rl + 1, 0:cl - 1],
                                      Pv[:, 0:rl, 1:cl], ADD)
                    ACT.copy(Ej3[:, :, 0:1], Pv[:, 0:rl, 0:1])
                    # E = E*lam + gamma_j ; E += P ; E += alpha'_j*S_j
                    DVE.tensor_scalar(Ej, Ej, LAM, g_ap[j], MULT, ADD)
                    DVE.tensor_tensor(Ej3[:, :, :], Ej3[:, :, :], Pd, ADD)
                    DVE.tensor_tensor(Ej, Ej, S[j][:, 0:rl * cl], ADD)

                # ---- EC = -E1 col-duplicated ----
                E13 = _v3(E[1][:, :], 320)
                E1dup = E13.unsqueeze(3).broadcast_to([128, 16, 320, 2])
                ECr = EC[:, :].rearrange("p (r cp two) -> p r cp two", r=16, two=2)
                ACT.mul(ECr[:, :, :, :], E1dup, -1.0)

                # ---- update: C = u + T*SQ1 ; u = C + EC(rowdup) ----
                DVE.tensor_scalar(T[:, :], T[:, :], SQ1, None, MULT)
                C = pool.tile([128, 32 * 640], F16, name=f"c{_step}", tag="scr")
                C3 = _v3(C, 640)
                DVE.tensor_tensor(C3[:, 0:26, :], uav[:, 1:27, :], T3[:, 0:26, :], ADD)
                GPS.tensor_tensor(C3[:, 26:32, :], uav[:, 27:33, :], T3[:, 26:32, :], ADD)
                Cp = C.rearrange("p (rp two c) -> p rp two c", rp=16, two=2, c=640)
                ur = uav[:, 1:33, :].rearrange("p (rp two) c -> p rp two c", rp=16, two=2)
                EC3 = _v3(EC[:, :], 640)
                DVE.tensor_tensor(ur[:, :, 0, :], Cp[:, :, 0, :], EC3, ADD)
                DVE.tensor_tensor(ur[:, :, 1, :], Cp[:, :, 1, :], EC3, ADD)

            # store + cast f16 -> f32
            GPS.dma_start(out_dram, uav[:, 1:33, :])

    nc.finalize()
    return nc


def _consts_np(bs: float, br: float) -> np.ndarray:
    c = np.zeros(8, np.float32)
    for j in range(1, 5):
        gam = j * br / DIAG
        if j == 1:
            gam += bs / DIAG
        c[j - 1] = np.float32(gam)
    c[4] = np.float32(5 * br / DIAG)
    return np.tile(c[None, :], (128, 1))


def kernel(u, b_smooth, b_res, t):
    t = int(np.asarray(t))
    u = np.ascontiguousarray(np.asarray(u, np.float32))
    bs = float(np.asarray(b_smooth).reshape(-1)[0])
    br = float(np.asarray(b_res).reshape(-1)[0])
    u2 = u[0, 0]

    if t not in _NC_CACHE:
        _NC_CACHE[t] = _build(t)
    nc = _NC_CACHE[t]

    consts = _consts_np(bs, br)
    in_maps = []
    for k in range(8):
        w = u2[:, 0:WCOL] if k == 0 else u2[:, OWN * k - HALO: OWN * k + OWN]
        in_maps.append({"u": np.ascontiguousarray(w), "consts": consts})

    res = run_bass_kernel_spmd(nc, in_maps, list(range(8)))
    outs = []
    for k in range(8):
        o = res.results[k]["out"]
        outs.append(o[:, 0:OWN] if k == 0 else o[:, HALO:WCOL])
    full = np.concatenate(outs, axis=1)
    return full[None, None].astype(np.float32)



# revision 7
# speedup vs baseline: 1.7224x; 1.7224x over previous
"""Multigrid F-cycle advection smoother on 8 TRN2 NeuronCores.

Strategy (zero-communication, redundant compute):
  - Column-shard: core k computes a [4096, 576] window of u:
    core 0 -> cols [0, 576), core k>0 -> cols [512k-64, 512k+512).
    Host keeps cols [0:512) from core 0, [64:576) from others.
    (Halo-64 validated vs reference: owned-col error 4e-5 in f32 sim.)
  - On-chip layout: partition p holds rows [32p, 32p+32); free = [row, col].
  - All intermediates f16. Raw (unscaled) block-sum restriction chain
    S1..S5; per-level constants folded:
      E_j = Pd + lam*(Pu0 + Pl0) + alpha_j*S_j + gamma_j
    with gamma_j injected by biasing the prolongation (P' = P + g'_j,
    g'_j = gamma_j/(1+2*lam)) so the chain is 3 fused STT ops per level.
  - Cross-partition halo rows (up-shift) via TensorE identity matmul with
    partition-offset PSUM views + ACT evacuation (no DMA on hot path).
  - Engines: DVE does all 2-input math; ACT does prolongation/EC/evac;
    TensorE does halos; GPS only does cast-DMAs; load/store chunked to
    overlap step-1 compute and step-4 store.
"""
import numpy as np

import concourse.bass as bass
import concourse.bacc as bacc
import concourse.mybir as mybir
from concourse import tile
from concourse.bass_utils import run_bass_kernel_spmd
from concourse.masks import make_identity

F32 = mybir.dt.float32
F16 = mybir.dt.float16
ADD = mybir.AluOpType.add
MULT = mybir.AluOpType.mult
COPY_F = mybir.ActivationFunctionType.Copy
IDENT_F = mybir.ActivationFunctionType.Identity

NROW, WCOL, OWN, HALO = 4096, 576, 512, 64
DIAG = 1.2
LAM = float(np.float32(0.1 / DIAG))
SQ1 = float(np.float32(0.1 / DIAG))
ALPHA = {j: float(np.float32(-0.1 * 4.0 ** (-j) / DIAG)) for j in range(1, 6)}

_NC_CACHE = {}


def _v3(ap, c):
    return ap.rearrange("p (r c) -> p r c", c=c)


def _build(t_steps: int):
    nc = bacc.Bacc(None, target_bir_lowering=False)
    u_ext = nc.declare_dram_parameter("u", [NROW, WCOL], F32, isOutput=False)
    c_ext = nc.declare_dram_parameter("consts", [128, 8], F32, isOutput=False)
    out_ext = nc.declare_dram_parameter("out", [NROW, WCOL], F32, isOutput=True)

    u_dram = u_ext[:, :].rearrange("(p r) c -> p r c", p=128)
    out_dram = out_ext[:, :].rearrange("(p r) c -> p r c", p=128)

    DVE, GPS, ACT = nc.vector, nc.gpsimd, nc.scalar

    LD = {j: (32 >> j, WCOL >> j) for j in range(1, 6)}  # per-partition rl, cl

    with tile.TileContext(nc) as tc:
        with tc.tile_pool(name="sb", bufs=1) as pool, \
             tc.tile_pool(name="ps", bufs=1, space="PSUM") as ppool:
            ua = pool.tile([128, 33 * WCOL], F16, name="ua")   # row 0 = up halo
            T = pool.tile([128, 32 * WCOL], F16, name="t")
            C = pool.tile([128, 32 * WCOL], F16, name="c")
            RP = pool.tile([128, 16 * WCOL], F16, name="rp")
            EC = pool.tile([128, 16 * WCOL], F16, name="ec")
            S = {1: pool.tile([128, 16 * (WCOL >> 1)], F16, name="s1")}
            RA = {}
            for j in range(2, 6):
                rl, cl = LD[j]
                dt = F32 if j >= 4 else F16
                RA[j] = pool.tile([128, max(rl * 2 * cl, 8)], dt, name=f"ra{j}")
                S[j] = pool.tile([128, max(rl * cl, 8)], dt, name=f"s{j}")
            E = {j: pool.tile([128, max(LD[j][0] * LD[j][1], 8)], F16, name=f"e{j}")
                 for j in range(1, 6)}
            P = {j: pool.tile([128, (1 + LD[j][0]) * (LD[j][1] + 2)], F16,
                              name=f"p{j}") for j in range(1, 5)}
            zsh = pool.tile([128, 130], F16, name="zsh")
            consts = pool.tile([128, 8], F32, name="k")
            psum = ppool.tile([128, 1024], F32, name="hps")

            nc.sync.dma_start(consts[:, :], c_ext[:, :])
            DVE.memset(zsh[:, :], 0.0)
            make_identity(nc, zsh[:, 1:129], nomemset=True)
            uav = _v3(ua[:, :], WCOL)
            # chunked load: 4 row-groups of 8 rows per partition
            for ch in range(4):
                GPS.dma_start(uav[:, 1 + 8 * ch:9 + 8 * ch, :],
                              u_dram[:, 8 * ch:8 + 8 * ch, :])

            gp_ap = {j: consts[:, j - 1:j] for j in range(1, 5)}
            gam5_ap = consts[:, 4:5]
            # presets: P zero-col (all rows) and partition-0 halo row := g'_j
            for j in range(1, 5):
                rl, cl = LD[j]
                Pv = _v3(P[j][:, :], cl + 2)
                zc = Pv[:, :, 0:1].rearrange("p r one -> p (r one)")
                DVE.memset(zc, 0.0)
                ACT.activation(zc, zc, IDENT_F, bias=gp_ap[j], scale=0.0)

            T3 = _v3(T[:, :], WCOL)
            u_data = ua[:, WCOL:]          # flat [128, 32*WCOL]
            EC3 = _v3(EC[:, :], WCOL)
            Tp = T[:, :].rearrange("p (rp two c) -> p rp two c", two=2, c=WCOL)
            RP3 = _v3(RP[:, :], WCOL)
            RPc = RP[:, :].rearrange("p (r cp two) -> p r cp two", r=16, two=2)
            S13 = _v3(S[1][:, :], WCOL >> 1)
            Cp = C[:, :].rearrange("p (rp two c) -> p rp two c", two=2, c=WCOL)
            ur = uav[:, 1:33, :].rearrange("p (rp two) c -> p rp two c", two=2)
            E13 = _v3(E[1][:, :], WCOL >> 1)

            def t_rows(r0, r1):
                """T = u_up + u_left for stripe rows [r0, r1)."""
                DVE.tensor_tensor(T3[:, r0:r1, 1:WCOL], uav[:, r0:r1, 1:WCOL],
                                  uav[:, r0 + 1:r1 + 1, 0:WCOL - 1], ADD)
                DVE.tensor_tensor(T3[:, r0:r1, 0:1], uav[:, r0:r1, 0:1],
                                  uav[:, r0 + 1:r1 + 1, 0:1], ADD)

            def c_rows(r0, r1):
                DVE.scalar_tensor_tensor(C[:, r0 * WCOL:r1 * WCOL],
                                         T[:, r0 * WCOL:r1 * WCOL], SQ1,
                                         u_data[:, r0 * WCOL:r1 * WCOL],
                                         MULT, ADD)

            def u_halo():
                nc.tensor.matmul(psum[:, 0:512], zsh[:, 0:128],
                                 uav[:, 32, 0:512])
                nc.tensor.matmul(psum[:, 512:WCOL], zsh[:, 0:128],
                                 uav[:, 32, 512:WCOL])
                ACT.activation(uav[:, 0, :], psum[:, 0:WCOL], COPY_F)
                ACT.copy(uav[0:1, 0, :], uav[0:1, 1, :])

            for _step in range(t_steps):
                first = _step == 0
                if first:
                    # overlap the load: rows 1.. of each chunk need no halo
                    t_rows(1, 8)
                    c_rows(1, 8)
                    for ch in range(1, 4):
                        t_rows(8 * ch, 8 * ch + 8)
                        c_rows(8 * ch, 8 * ch + 8)
                    u_halo()           # waits only on last-loaded chunk
                    t_rows(0, 1)
                    c_rows(0, 1)
                else:
                    u_halo()
                    t_rows(0, 32)

                # ---- restriction: RP = rowpair(T); S1 = colpair(RP) ----
                DVE.tensor_tensor(RP3[:, :, :], Tp[:, :, 0, :], Tp[:, :, 1, :],
                                  ADD)
                DVE.tensor_tensor(S13[:, :, :], RPc[:, :, :, 0],
                                  RPc[:, :, :, 1], ADD)
                prev = S[1]
                for j in range(2, 6):
                    rl, cl = LD[j]
                    pv = prev[:, 0:rl * 4 * cl].rearrange(
                        "p (rp two c) -> p rp two c", two=2, c=2 * cl)
                    ra3 = _v3(RA[j][:, 0:rl * 2 * cl], 2 * cl)
                    DVE.tensor_tensor(ra3[:, :, :], pv[:, :, 0, :],
                                      pv[:, :, 1, :], ADD)
                    rac = RA[j][:, 0:rl * 2 * cl].rearrange(
                        "p (r cp two) -> p r cp two", r=rl, two=2)
                    s3 = _v3(S[j][:, 0:rl * cl], cl)
                    DVE.tensor_tensor(s3[:, :, :], rac[:, :, :, 0],
                                      rac[:, :, :, 1], ADD)
                    prev = S[j]

                # ---- E5 = alpha5*S5 + gamma5 ----
                rl5, cl5 = LD[5]
                DVE.tensor_scalar(E[5][:, 0:cl5], S[5][:, 0:cl5], ALPHA[5],
                                  gam5_ap, MULT, ADD)

                if not first:
                    c_rows(0, 16)   # overlaps ACT prolongation of levels 4..3

                # ---- up chain j=4..1 ----
                for j in range(4, 0, -1):
                    rl, cl = LD[j]
                    r2, c2 = rl // 2, cl // 2
                    Pv = _v3(P[j][:, :], cl + 2)
                    Ein3 = _v3(E[j + 1][:, 0:r2 * c2], c2)
                    # prolongation with gamma bias: Pdata = dup(E_{j+1}) + g'
                    Pr = Pv[:, 1:rl + 1, 1:cl + 1].rearrange(
                        "p (r2 a) c -> p r2 a c", a=2).rearrange(
                        "p r2 a (c2 b) -> p r2 a c2 b", b=2)
                    Edup = Ein3.unsqueeze(3).broadcast_to([128, r2, c2, 2])
                    ACT.activation(Pr[:, :, 0, :, :], Edup, IDENT_F,
                                   bias=gp_ap[j])
                    ACT.activation(Pr[:, :, 1, :, :], Edup, IDENT_F,
                                   bias=gp_ap[j])
                    # halo row: prev partition's last P data row, from E_{j+1}
                    Elast = Ein3[:, r2 - 1, :].unsqueeze(2).broadcast_to(
                        [128, c2, 2])
                    nc.tensor.matmul(psum[:, 0:cl], zsh[:, 0:128], Elast)
                    ACT.activation(Pv[:, 0, 1:cl + 1], psum[:, 0:cl],
                                   IDENT_F, bias=gp_ap[j])
                    # chain: E_j = Pd + lam*(Pu0+Pl0) + alpha_j*S_j (+gamma)
                    Ej = E[j][:, 0:rl * cl]
                    Ej3 = _v3(Ej, cl)
                    DVE.scalar_tensor_tensor(Ej3, Pv[:, 0:rl, 1:cl + 1], LAM,
                                             Pv[:, 1:rl + 1, 1:cl + 1], MULT,
                                             ADD)
                    DVE.scalar_tensor_tensor(Ej, S[j][:, 0:rl * cl], ALPHA[j],
                                             Ej, MULT, ADD)
                    DVE.scalar_tensor_tensor(Ej3, Pv[:, 1:rl + 1, 0:cl], LAM,
                                             Ej3, MULT, ADD)
                    if j == 2 and not first:
                        c_rows(16, 32)   # overlaps prolongation of level 1

                # ---- EC = -E1 (col-dup) + final update, split by row halves
                last = _step == t_steps - 1
                for rh in range(2):
                    q0, q1 = 8 * rh, 8 * rh + 8
                    E1dup = E13[:, q0:q1, :].unsqueeze(3).broadcast_to(
                        [128, 8, WCOL >> 1, 2])
                    ECr = EC3[:, q0:q1, :].rearrange(
                        "p r (cp two) -> p r cp two", two=2)
                    ACT.mul(ECr, E1dup, -1.0)
                    DVE.tensor_tensor(ur[:, q0:q1, 0, :], Cp[:, q0:q1, 0, :],
                                      EC3[:, q0:q1, :], ADD)
                    DVE.tensor_tensor(ur[:, q0:q1, 1, :], Cp[:, q0:q1, 1, :],
                                      EC3[:, q0:q1, :], ADD)
                    if last:
                        GPS.dma_start(out_dram[:, 16 * rh:16 * rh + 8, :],
                                      uav[:, 1 + 16 * rh:9 + 16 * rh, :])
                        GPS.dma_start(out_dram[:, 16 * rh + 8:16 * rh + 16, :],
                                      uav[:, 9 + 16 * rh:17 + 16 * rh, :])

    nc.finalize()
    return nc


def _consts_np(bs: float, br: float) -> np.ndarray:
    c = np.zeros(8, np.float32)
    for j in range(1, 5):
        gam = j * br / DIAG
        if j == 1:
            gam += bs / DIAG
        c[j - 1] = np.float32(gam / (1.0 + 2.0 * LAM))
    c[4] = np.float32(5 * br / DIAG)
    return np.tile(c[None, :], (128, 1))


def kernel(u, b_smooth, b_res, t):
    t = int(np.asarray(t))
    u = np.ascontiguousarray(np.asarray(u, np.float32))
    bs = float(np.asarray(b_smooth).reshape(-1)[0])
    br = float(np.asarray(b_res).reshape(-1)[0])
    u2 = u[0, 0]

    if t not in _NC_CACHE:
        _NC_CACHE[t] = _build(t)
    nc = _NC_CACHE[t]

    consts = _consts_np(bs, br)
    in_maps = []
    for k in range(8):
        w = u2[:, 0:WCOL] if k == 0 else u2[:, OWN * k - HALO: OWN * k + OWN]
        in_maps.append({"u": np.ascontiguousarray(w), "consts": consts})

    res = run_bass_kernel_spmd(nc, in_maps, list(range(8)))
    outs = []
    for k in range(8):
        o = res.results[k]["out"]
        outs.append(o[:, 0:OWN] if k == 0 else o[:, HALO:WCOL])
    full = np.concatenate(outs, axis=1)
    return full[None, None].astype(np.float32)
